# revision 55
# baseline (speedup 1.0000x reference)
"""CFG sub-AST expression combiner (segment-softmax scatter attention) on 8 trn2 cores.

Strategy: sort edges by segment (PDG node); windows of W=16 segments assigned
round-robin by descending edge count (load balance, softmax core-local). Host
folds Wq/Wk into a per-segment vector table C = A @ (Wq Wk^T)/sqrt(d) and
pre-gathers edge value rows into two DRAM layouts (slot-major V and transposed
V^T) so the device streams everything with large contiguous DMAs. Pipeline:
scores per 4-tile group = V^T @ C (PE, one PSUM bank per group, 5-deep ring)
-> exp per group (ACT) -> mask multiply per group (DVE, ~2/5 offloaded to the
otherwise-idle GPSIMD) -> per window pair: all-ones denominator matmuls then
PV matmuls into a shared PSUM bank (reciprocal overlaps the PV streams) ->
normalize pv*(1/dn) (DVE) -> project with Wo every 16 windows (PE) -> bias add
via ACT Identity (per-partition bias). Empty segments get bias-only on host.
PE cost here is ldweights-bound (~72ns per matmul regardless of rows), so the
design minimizes matmul COUNT; narrower matmuls measured slower (DVFS).
"""

import math

import numpy as np
import ml_dtypes

import concourse.bass as bass
from concourse import bacc
import concourse.mybir as mybir
from concourse.bass_types import AP
from concourse.tile import TileContext, add_dep_helper
from concourse import bass_utils

BF16 = ml_dtypes.bfloat16
N_CORES = 8
D = 128          # feature dim
H = 8            # heads
W = 16           # segment window width (output columns per score matmul)
HW = H * W       # score columns per tile (128)
P = 128          # edge slots per tile (partition dim)
PROJ_B = 16      # windows per output-projection batch (PROJ_B*W = 256 cols)
TC = 32          # max tiles per chunk
GRP = 4          # tiles per score group (= 1 PSUM bank)
AHEAD = 3        # score-group lookahead (ps_s bufs = AHEAD + 1)
F32 = mybir.dt.float32
BF = mybir.dt.bfloat16


def _build_nc(NW, T_ws, chunks, comb, wlo, whi):
    """One SPMD program for all cores. NW windows of W segments; window j owns
    T_ws[j] tiles of P edge slots (uniform across cores). chunks: list of
    (j0, j1, o0, o1) slot/tile ranges streamed together (j0/j1 even).
    wlo/whi: per global tile, the union (over cores) of covered segments
    within the window -- score/pv/dn matmuls narrow to that column span.
    Score columns are (w, h)-major so a segment span is contiguous."""
    S_pad = NW * W
    S_t = sum(T_ws)
    assert NW % PROJ_B == 0 and NW % 2 == 0
    nc = bacc.Bacc("TRN2", target_bir_lowering=False)

    ev = nc.dram_tensor("ev", [P, S_t * D], BF, kind="ExternalInput")
    evt = nc.dram_tensor("evt", [D, S_t * P], BF, kind="ExternalInput")
    cc = nc.dram_tensor("cc", [D, NW * HW], BF, kind="ExternalInput")
    msk = nc.dram_tensor("msk", [P, S_t * W], BF, kind="ExternalInput")
    wo = nc.dram_tensor("wo", [D, H * comb], BF, kind="ExternalInput")
    bo = nc.dram_tensor("bo", [comb, 1], F32, kind="ExternalInput")
    out = nc.dram_tensor("out", [comb, S_pad], BF, kind="ExternalOutput")

    EXP = mybir.ActivationFunctionType.Exp
    IDNT = mybir.ActivationFunctionType.Identity

    tile_off = [0]
    for t in T_ws:
        tile_off.append(tile_off[-1] + t)

    # window -> (chunk k, chunk-local tile offset, chunk-local window idx)
    geom = [None] * NW
    for k, (j0, j1, o0, o1) in enumerate(chunks):
        ol = 0
        for j in range(j0, j1):
            geom[j] = (k, ol, j - j0)
            ol += T_ws[j]

    # score groups: <=GRP consecutive tiles, never crossing a chunk boundary
    groups = []            # (k, ol0, gsz)
    grp_of_tile = {}       # global tile idx -> (group idx, slot in group)
    for k, (j0, j1, o0, o1) in enumerate(chunks):
        ol = 0
        while ol < o1 - o0:
            gsz = min(GRP, o1 - o0 - ol)
            gi = len(groups)
            for s in range(gsz):
                grp_of_tile[o0 + ol + s] = (gi, s)
            groups.append((k, ol, gsz))
            ol += gsz
    NG = len(groups)

    # window of each tile (for score rhs selection)
    win_of_tile = {}
    for j in range(NW):
        for t in range(T_ws[j]):
            win_of_tile[tile_off[j] + t] = j

    with TileContext(nc) as tc:
        with (
            tc.tile_pool(name="const", bufs=1) as constp,
            tc.tile_pool(name="ccp", bufs=4) as ccp,
            tc.tile_pool(name="mkp", bufs=4) as mkp,
            tc.tile_pool(name="vg", bufs=4) as vgp,
            tc.tile_pool(name="vt", bufs=4) as vtp,
            tc.tile_pool(name="sx", bufs=AHEAD + 2) as sxp,
            tc.tile_pool(name="pt", bufs=AHEAD + 2) as ptp,
            tc.tile_pool(name="hot", bufs=2) as hotp,
            tc.tile_pool(name="rec", bufs=2) as recp,
            tc.tile_pool(name="ps_s", bufs=AHEAD + 1, space="PSUM") as ps_s,
            tc.tile_pool(name="ps_acc", bufs=2, space="PSUM") as ps_acc,
        ):
            ones_mat = constp.tile([P, P], BF, tag="ones_mat")
            nc.vector.memset(ones_mat[:], 1.0)
            outb = constp.tile([comb, S_pad], BF, tag="outb")
            # zero the score-PSUM ring once: narrow score matmuls leave
            # never-written columns, whose initial content would otherwise be
            # arbitrary (exp of garbage can be Inf; Inf*0 mask = NaN). After
            # warm-up the stale columns hold old finite scores, which is fine.
            for _ in range(AHEAD + 1):
                zt = ps_s.tile([P, GRP, HW], F32, tag="s", name="s_init")
                nc.vector.memset(zt[:], 0.0)

            n_ch = len(chunks)
            G_MAX = max(j1 - j0 for j0, j1, _, _ in chunks)
            cc_t = [None] * n_ch
            mk_t = [None] * n_ch
            vg_t = [None] * n_ch
            vt_t = [None] * n_ch
            pt_t = [None] * NG
            sx_t = [None] * NG

            state = {"issued": 0, "xped": 0, "scored": 0}

            def issue_chunk(k):
                j0, j1, o0, o1 = chunks[k]
                Tc = o1 - o0
                vg_t[k] = vgp.tile([P, TC * D], BF, tag="vg", name="vgt")
                nc.sync.dma_start(vg_t[k][:, 0:Tc * D], ev[:, o0 * D:o1 * D])
                vt_t[k] = vtp.tile([D, TC * P], BF, tag="vt", name="vtt")
                nc.sync.dma_start(vt_t[k][:, 0:Tc * P], evt[:, o0 * P:o1 * P])
                mk_t[k] = mkp.tile([P, TC * W], BF, tag="mk", name="mkt")
                nc.sync.dma_start(mk_t[k][:, 0:Tc * W], msk[:, o0 * W:o1 * W])
                cc_t[k] = ccp.tile([D, G_MAX * HW], BF, tag="cc", name="cct")
                G = j1 - j0
                nc.sync.dma_start(cc_t[k][:, 0:G * HW], cc[:, j0 * HW:j1 * HW])

            def ensure_chunk(k):
                while state["issued"] <= k and state["issued"] < n_ch:
                    issue_chunk(state["issued"])
                    state["issued"] += 1

            def emit_score_group(g):
                k, ol0, gsz = groups[g]
                j0, j1, o0, o1 = chunks[k]
                ensure_chunk(k + 1)
                s_ps = ps_s.tile([P, GRP, HW], F32, tag="s", name="s_ps")
                for s in range(gsz):
                    gt = o0 + ol0 + s
                    j = win_of_tile[gt]
                    jc = j - j0
                    tloc = ol0 + s
                    nc.tensor.matmul(
                        s_ps[:, s, :],
                        lhsT=vt_t[k][:, tloc * P:(tloc + 1) * P],
                        rhs=cc_t[k][:, jc * HW:(jc + 1) * HW],
                        start=True,
                        stop=True,
                    )
                sx = sxp.tile([P, GRP, H, W], BF, tag="sx", name="sx")
                sx_t[g] = sx
                nc.scalar.activation(
                    sx[:, 0:gsz].rearrange("p t h w -> p t (h w)"),
                    s_ps[:, 0:gsz, :],
                    EXP,
                )
                pt = ptp.tile([P, GRP, H, W], BF, tag="pt", name="pt")
                pt_t[g] = pt

                def mask_bcast(t0, t1):
                    mv = mk_t[k][:, (ol0 + t0) * W:(ol0 + t1) * W].rearrange(
                        "p (t w) -> p t w", w=W)
                    return AP(mv.tensor, mv.offset,
                              [mv.ap[0], mv.ap[1], [0, H], mv.ap[2]])

                # 3/5 of mask multiplies on the otherwise-idle GPSIMD
                # (~1.15us/group, max run of 2) and 2/5 on DVE (~0.45us):
                # fewer DVE masks means fewer get head-of-line blocked behind
                # the big quad-normalize ops that PE then waits on
                eng = nc.gpsimd if g % 5 in (0, 1, 3) else nc.vector
                eng.tensor_mul(pt[:, 0:gsz], sx[:, 0:gsz],
                               mask_bcast(0, gsz))

            def ensure_scores(g_need):
                tgt = min(g_need + AHEAD, NG - 1)
                while state["scored"] <= tgt:
                    emit_score_group(state["scored"])
                    state["scored"] += 1

            hot = None

            def emit_quad(q):
                nonlocal hot
                wins = [4 * q + i for i in range(4)]
                last_tile = tile_off[wins[3]] + T_ws[wins[3]] - 1
                ensure_scores(grp_of_tile[last_tile][0])

                # quad accumulator: pv of 4 windows in bank0, dn in bank1
                acc = ps_acc.tile([P, 8 * HW], F32, tag="acc", name="acc")

                def pt_slice(gt):
                    g, s = grp_of_tile[gt]
                    return pt_t[g][:, s, :, :].rearrange("p h w -> p (h w)")

                # dn groups first: the reciprocal then runs on DVE while the
                # pv matmuls are still streaming on PE
                dn_last = None
                for wi, j in enumerate(wins):
                    base = (4 + wi) * HW
                    for t in range(T_ws[j]):
                        mm = nc.tensor.matmul(
                            acc[:, base:base + HW],
                            lhsT=ones_mat[:],
                            rhs=pt_slice(tile_off[j] + t),
                            start=(t == 0),
                            stop=(t == T_ws[j] - 1),
                        )
                        if t == 0 and dn_last is not None:
                            add_dep_helper(mm.ins, dn_last.ins,
                                           reason="dn group order in shared bank")
                        dn_last = mm
                rec = recp.tile([P, 4 * HW], F32, tag="rec", name="rec")
                nc.vector.reciprocal_approx_fast(rec[:], acc[:, 4 * HW:8 * HW])
                pv_last = None
                for wi, j in enumerate(wins):
                    k, ol, jc = geom[j]
                    base = wi * HW
                    for t in range(T_ws[j]):
                        mm = nc.tensor.matmul(
                            acc[:, base:base + HW],
                            lhsT=vg_t[k][:, (ol + t) * D:(ol + t + 1) * D],
                            rhs=pt_slice(tile_off[j] + t),
                            start=(t == 0),
                            stop=(t == T_ws[j] - 1),
                        )
                        if t == 0 and pv_last is not None:
                            add_dep_helper(mm.ins, pv_last.ins,
                                           reason="pv group order in shared bank")
                        pv_last = mm
                # normalize hot = pv * (1/dn) for all 4 windows at once
                qq = q % (PROJ_B // 4)
                if qq == 0:
                    hot = hotp.tile([P, PROJ_B, H, W], BF, tag="hot",
                                    name="hot")
                hslc = hot[:, qq * 4:(qq + 1) * 4].rearrange(
                    "p b h w -> p (b h w)")
                nc.vector.tensor_mul(hslc, acc[:, 0:4 * HW], rec[:])
                # output projection every PROJ_B windows
                if qq == PROJ_B // 4 - 1:
                    jbase = (q + 1) * 4 - PROJ_B
                    # ops borrows a score-PSUM slot (2 of its 4 tile slots)
                    ops = ps_s.tile([P, GRP, HW], F32, tag="s",
                                    name="ops")[:, 0:2, :].rearrange(
                                        "c b w -> c (b w)")
                    for h in range(H):
                        nc.tensor.matmul(
                            ops.rearrange("c (b w) -> c b w", b=PROJ_B),
                            lhsT=wo_sb[:, h, :],
                            rhs=hot[:, :, h, :],
                            start=(h == 0),
                            stop=(h == H - 1),
                        )
                    nc.scalar.activation(
                        outb[:, jbase * W:(jbase + PROJ_B) * W], ops, IDNT,
                        bias=bo_sb[:],
                    )

            ensure_chunk(1)
            # constants after the first chunks so they don't delay the pipe
            wo_sb = constp.tile([D, H, comb], BF, tag="wo")
            nc.sync.dma_start(wo_sb[:], wo[:].rearrange("d (h c) -> d h c", h=H))
            bo_sb = constp.tile([comb, 1], F32, tag="bo")
            nc.sync.dma_start(bo_sb[:], bo[:])
            for q in range(NW // 4):
                emit_quad(q)

            nc.sync.dma_start(out[:], outb[:])
    nc.compile()
    return nc


def _plan(ast_value, N):
    """Window/tile structure + per-edge slot assignment (core, partition, tile)."""
    E = ast_value.shape[0]
    NWg = -(-N // W)               # global window count
    order = np.argsort(ast_value, kind="stable")
    seg_s = ast_value[order].astype(np.int64)
    win_s = seg_s // W

    n_w = np.bincount(win_s, minlength=NWg)
    t_w = np.maximum(1, -(-n_w // P))
    # round-robin by descending edge count -> near-equal per-core tile budgets
    wrank = np.argsort(-n_w, kind="stable")
    core_of_w = np.empty(NWg, np.int64)
    core_of_w[wrank] = np.arange(NWg) % N_CORES
    rank_in_core = np.arange(NWg) // N_CORES    # per wrank order, desc sizes
    NW = -(-NWg // N_CORES)
    NW = -(-NW // PROJ_B) * PROJ_B
    # pair big with small: per-core rank r -> slot 2*min(r, NW-1-r) + side
    r = np.arange(NW)
    slot_of_rank = np.where(r < NW - 1 - r, 2 * r, 2 * (NW - 1 - r) + 1)
    slot_of_w = np.empty(NWg, np.int64)
    slot_of_w[wrank] = slot_of_rank[rank_in_core]
    # shared (max-over-octet) tile counts per slot; wrank desc => rank 8j max
    T_rank = np.ones(NW, np.int64)
    n_full = (NWg + N_CORES - 1) // N_CORES
    T_rank[:n_full] = t_w[wrank[0::N_CORES]]
    T_ws = np.empty(NW, np.int64)
    T_ws[slot_of_rank] = T_rank
    tile_off = np.zeros(NW + 1, np.int64)
    np.cumsum(T_ws, out=tile_off[1:])
    S_t = int(tile_off[-1])

    starts = np.zeros(NWg, np.int64)
    np.cumsum(n_w[:-1], out=starts[1:])
    rank_e = np.arange(E, dtype=np.int64) - starts[win_s]
    t_e = rank_e // P
    p_e = rank_e % P
    core_e = core_of_w[win_s]
    g_e = tile_off[slot_of_w[win_s]] + t_e
    return (order, seg_s, win_s, core_of_w, slot_of_w, NW, T_ws, tile_off,
            S_t, core_e, p_e, g_e)


def _run(ast, Wq, bq, Wk, bk, Wo, bo, ast_key, ast_value, pdg_key, pdg_value, N,
         trace=False):
    """Host orchestration: build plan from data, compile, run on 8 cores."""
    n_tbl, d = ast.shape
    assert d == D
    comb = Wo.shape[1]
    sc = 1.0 / math.sqrt(D)

    (order, seg_s, win_s, core_of_w, slot_of_w, NW, T_ws, tile_off, S_t,
     core_e, p_e, g_e) = _plan(ast_value, N)
    key_s = ast_key[order].astype(np.int64)
    NWg = -(-N // W)

    # host pre-gather: slot (core, p, tile) -> value row, in both layouts.
    # unused slots point at a zero row.
    tblz = np.vstack([ast.astype(BF16), np.zeros((1, D), BF16)])
    gidx_all = np.full((N_CORES, P, S_t), n_tbl, np.int64)
    gidx_all[core_e, p_e, g_e] = key_s
    ev_all = tblz[gidx_all.reshape(N_CORES, -1)]          # [8, P*S_t, D]
    ev_all = ev_all.reshape(N_CORES, P, S_t * D)
    evt_all = np.ascontiguousarray(
        ev_all.reshape(N_CORES, P, S_t, D).transpose(0, 3, 2, 1)
    ).reshape(N_CORES, D, S_t * P)

    # multiplicative mask
    msk_f = np.zeros((N_CORES, P, S_t * W), np.float32)
    msk_f[core_e, p_e, g_e * W + seg_s % W] = 1.0
    msk_all = msk_f.astype(BF16)

    # ---- query-side fold: C = A @ (Wq' Wk^T) + bq' @ Wk^T ----
    qsrc = np.zeros(N, np.int64)
    qsrc[pdg_key.astype(np.int64)] = pdg_value.astype(np.int64)
    A = ast[qsrc]                                        # [N, D] f32
    M = np.einsum("hij,hkj->hik", Wq * sc, Wk)           # [H, D, D]
    kap = np.einsum("hj,hkj->hk", bq * sc, Wk)           # [H, D]
    C8 = np.einsum("nd,hdk->hnk", A, M) + kap[:, None, :]  # [H, N, D]

    # per-core window lists -> cc layout [D, NW*H*W]
    wl = np.full((N_CORES, NW), -1, np.int64)
    wl[core_of_w, slot_of_w] = np.arange(NWg)
    seg_raw = wl[:, :, None] * W + np.arange(W)[None, None, :]  # [8, NW, W]
    valid = (wl[:, :, None] >= 0) & (seg_raw < N)
    seg_ids = np.clip(seg_raw, 0, N - 1)
    ccv = C8[:, seg_ids, :]                              # [H, 8, NW, W, D]
    cc_all = np.ascontiguousarray(
        ccv.transpose(1, 4, 2, 0, 3)                     # [8, D, NW, H, W]
    ).astype(BF16).reshape(N_CORES, D, NW * HW)

    # narrow-span matmuls measured SLOWER on hw (ldweights is the per-matmul
    # floor and short streams crater the DVFS duty cycle) -- full width
    wlo_t = np.zeros(S_t, np.int64)
    whi_t = np.full(S_t, W, np.int64)

    # chunks of consecutive slots with <= TC tiles, pair-aligned boundaries.
    # first chunks are small so the compute pipeline starts sooner.
    chunks = []
    j0 = 0
    tc_sched = []
    while j0 < NW:
        lim = tc_sched[len(chunks)] if len(chunks) < len(tc_sched) else TC
        j1 = j0
        while j1 < NW and tile_off[j1 + 1] - tile_off[j0] <= lim:
            j1 += 1
        if j1 < NW and (j1 - j0) % 2 == 1 and j1 - j0 >= 2:
            j1 -= 1
        chunks.append((j0, j1, int(tile_off[j0]), int(tile_off[j1])))
        j0 = j1

    wo_arr = np.ascontiguousarray(
        Wo.reshape(H, D, comb).transpose(1, 0, 2)
    ).astype(BF16).reshape(D, H * comb)
    bo_col = bo.reshape(comb, 1).astype(np.float32)

    nc = _build_nc(NW, [int(x) for x in T_ws], chunks, comb,
                   [int(x) for x in wlo_t], [int(x) for x in whi_t])
    in_maps = []
    for c in range(N_CORES):
        in_maps.append({
            "ev": ev_all[c],
            "evt": evt_all[c],
            "cc": cc_all[c],
            "msk": msk_all[c],
            "wo": wo_arr,
            "bo": bo_col,
        })
    res = bass_utils.run_bass_kernel_spmd(
        nc, in_maps, core_ids=list(range(N_CORES)), trace=trace
    )
    full = np.zeros((N, comb), np.float32)
    for c in range(N_CORES):
        outc = np.asarray(res.results[c]["out"]).astype(np.float32).T  # [S_pad, comb]
        vm = valid[c].reshape(-1)
        sel = seg_ids[c].reshape(-1)[vm]
        full[sel] = outc[: vm.shape[0]][vm]
    # empty segments: reference = bias only (p/0 is undefined there)
    seg_cnt = np.bincount(ast_value.astype(np.int64), minlength=N)[:N]
    full[seg_cnt == 0] = bo[None, :]
    return full, res


def kernel(**inputs):
    ast = np.asarray(inputs["ast_nodes_encodings"], np.float32)
    Wq = np.asarray(inputs["Wq"], np.float32)
    bq = np.asarray(inputs["bq"], np.float32)
    Wk = np.asarray(inputs["Wk"], np.float32)
    bk = np.asarray(inputs["bk"], np.float32)  # cancels inside segment softmax
    Wo = np.asarray(inputs["Wo"], np.float32)
    bo = np.asarray(inputs["bo"], np.float32)
    ast_key = np.asarray(inputs["ast_key"]).astype(np.int64)
    ast_value = np.asarray(inputs["ast_value"]).astype(np.int64)
    pdg_key = np.asarray(inputs["pdg_key"]).astype(np.int64)
    pdg_value = np.asarray(inputs["pdg_value"]).astype(np.int64)
    N = int(np.asarray(inputs["nr_cfg_nodes"]))
    out, _ = _run(ast, Wq, bq, Wk, bk, Wo, bo,
                  ast_key, ast_value, pdg_key, pdg_value, N)
    return out


# revision 57
# speedup vs baseline: 1.0458x; 1.0458x over previous
"""CFG sub-AST expression combiner (segment-softmax scatter attention) on 8 trn2 cores.

Strategy: sort edges by segment (PDG node); windows of W=16 segments assigned
round-robin by descending edge count (load balance, softmax core-local). Host
folds Wq/Wk into a per-segment vector table C = A @ (Wq Wk^T)/sqrt(d) and
pre-gathers edge value rows into two DRAM layouts (slot-major V and transposed
V^T) so the device streams everything with large contiguous DMAs. Pipeline:
scores per 4-tile group = V^T @ C (PE, one PSUM bank per group, 5-deep ring)
-> exp per group (ACT) -> mask multiply per group (DVE, ~2/5 offloaded to the
otherwise-idle GPSIMD) -> per window pair: all-ones denominator matmuls then
PV matmuls into a shared PSUM bank (reciprocal overlaps the PV streams) ->
normalize pv*(1/dn) (DVE) -> project with Wo every 16 windows (PE) -> bias add
via ACT Identity (per-partition bias). Empty segments get bias-only on host.
PE cost here is ldweights-bound (~72ns per matmul regardless of rows), so the
design minimizes matmul COUNT; narrower matmuls measured slower (DVFS).
"""

import math

import numpy as np
import ml_dtypes

import concourse.bass as bass
from concourse import bacc
import concourse.mybir as mybir
from concourse.bass_types import AP
from concourse.tile import TileContext, add_dep_helper
from concourse import bass_utils

BF16 = ml_dtypes.bfloat16
N_CORES = 8
D = 128          # feature dim
H = 8            # heads
W = 16           # segment window width (output columns per score matmul)
HW = H * W       # score columns per tile (128)
P = 128          # edge slots per tile (partition dim)
PROJ_B = 16      # windows per output-projection batch (PROJ_B*W = 256 cols)
TC = 32          # max tiles per chunk
GRP = 4          # tiles per score group (= 1 PSUM bank)
AHEAD = 3        # score-group lookahead (ps_s bufs = AHEAD + 1)
F32 = mybir.dt.float32
BF = mybir.dt.bfloat16


def _build_nc(NW, T_ws, chunks, comb, wlo, whi):
    """One SPMD program for all cores. NW windows of W segments; window j owns
    T_ws[j] tiles of P edge slots (uniform across cores). chunks: list of
    (j0, j1, o0, o1) slot/tile ranges streamed together (j0/j1 even).
    wlo/whi: per global tile, the union (over cores) of covered segments
    within the window -- score/pv/dn matmuls narrow to that column span.
    Score columns are (w, h)-major so a segment span is contiguous."""
    S_pad = NW * W
    S_t = sum(T_ws)
    assert NW % PROJ_B == 0 and NW % 2 == 0
    nc = bacc.Bacc("TRN2", target_bir_lowering=False)

    ev = nc.dram_tensor("ev", [P, S_t * D], BF, kind="ExternalInput")
    evt = nc.dram_tensor("evt", [D, S_t * P], BF, kind="ExternalInput")
    cc = nc.dram_tensor("cc", [D, NW * HW], BF, kind="ExternalInput")
    msk = nc.dram_tensor("msk", [P, S_t * W], BF, kind="ExternalInput")
    wo = nc.dram_tensor("wo", [D, H * comb], BF, kind="ExternalInput")
    bo = nc.dram_tensor("bo", [comb, 1], F32, kind="ExternalInput")
    out = nc.dram_tensor("out", [comb, S_pad], BF, kind="ExternalOutput")

    EXP = mybir.ActivationFunctionType.Exp
    IDNT = mybir.ActivationFunctionType.Identity

    tile_off = [0]
    for t in T_ws:
        tile_off.append(tile_off[-1] + t)

    # window -> (chunk k, chunk-local tile offset, chunk-local window idx)
    geom = [None] * NW
    for k, (j0, j1, o0, o1) in enumerate(chunks):
        ol = 0
        for j in range(j0, j1):
            geom[j] = (k, ol, j - j0)
            ol += T_ws[j]

    # score groups: <=GRP consecutive tiles, never crossing a chunk boundary
    groups = []            # (k, ol0, gsz)
    grp_of_tile = {}       # global tile idx -> (group idx, slot in group)
    for k, (j0, j1, o0, o1) in enumerate(chunks):
        ol = 0
        while ol < o1 - o0:
            gsz = min(GRP, o1 - o0 - ol)
            gi = len(groups)
            for s in range(gsz):
                grp_of_tile[o0 + ol + s] = (gi, s)
            groups.append((k, ol, gsz))
            ol += gsz
    NG = len(groups)

    # window of each tile (for score rhs selection)
    win_of_tile = {}
    for j in range(NW):
        for t in range(T_ws[j]):
            win_of_tile[tile_off[j] + t] = j

    with TileContext(nc) as tc:
        with (
            tc.tile_pool(name="const", bufs=1) as constp,
            tc.tile_pool(name="ccp", bufs=4) as ccp,
            tc.tile_pool(name="mkp", bufs=4) as mkp,
            tc.tile_pool(name="vg", bufs=4) as vgp,
            tc.tile_pool(name="vt", bufs=4) as vtp,
            tc.tile_pool(name="sx", bufs=AHEAD + 2) as sxp,
            tc.tile_pool(name="pt", bufs=AHEAD + 2) as ptp,
            tc.tile_pool(name="hot", bufs=2) as hotp,
            tc.tile_pool(name="rec", bufs=2) as recp,
            tc.tile_pool(name="ps_s", bufs=AHEAD + 1, space="PSUM") as ps_s,
            tc.tile_pool(name="ps_acc", bufs=2, space="PSUM") as ps_acc,
        ):
            ones_mat = constp.tile([P, P], BF, tag="ones_mat")
            nc.vector.memset(ones_mat[:], 1.0)
            outb = constp.tile([comb, S_pad], BF, tag="outb")
            # zero the score-PSUM ring once: narrow score matmuls leave
            # never-written columns, whose initial content would otherwise be
            # arbitrary (exp of garbage can be Inf; Inf*0 mask = NaN). After
            # warm-up the stale columns hold old finite scores, which is fine.
            for _ in range(AHEAD + 1):
                zt = ps_s.tile([P, GRP, HW], F32, tag="s", name="s_init")
                nc.vector.memset(zt[:], 0.0)

            n_ch = len(chunks)
            G_MAX = max(j1 - j0 for j0, j1, _, _ in chunks)
            cc_t = [None] * n_ch
            mk_t = [None] * n_ch
            vg_t = [None] * n_ch
            vt_t = [None] * n_ch
            pt_t = [None] * NG
            sx_t = [None] * NG

            state = {"issued": 0, "xped": 0, "scored": 0}

            def issue_chunk(k):
                j0, j1, o0, o1 = chunks[k]
                Tc = o1 - o0
                # score-path operands (vt, cc) first: the next chunk's
                # scores unblock ~3us earlier; vg (PV side) is needed last
                vt_t[k] = vtp.tile([D, TC * P], BF, tag="vt", name="vtt")
                nc.sync.dma_start(vt_t[k][:, 0:Tc * P], evt[:, o0 * P:o1 * P])
                cc_t[k] = ccp.tile([D, G_MAX * HW], BF, tag="cc", name="cct")
                G = j1 - j0
                nc.sync.dma_start(cc_t[k][:, 0:G * HW], cc[:, j0 * HW:j1 * HW])
                mk_t[k] = mkp.tile([P, TC * W], BF, tag="mk", name="mkt")
                nc.sync.dma_start(mk_t[k][:, 0:Tc * W], msk[:, o0 * W:o1 * W])
                vg_t[k] = vgp.tile([P, TC * D], BF, tag="vg", name="vgt")
                nc.sync.dma_start(vg_t[k][:, 0:Tc * D], ev[:, o0 * D:o1 * D])

            def ensure_chunk(k):
                while state["issued"] <= k and state["issued"] < n_ch:
                    issue_chunk(state["issued"])
                    state["issued"] += 1

            def emit_score_group(g):
                k, ol0, gsz = groups[g]
                j0, j1, o0, o1 = chunks[k]
                ensure_chunk(k + 1)
                s_ps = ps_s.tile([P, GRP, HW], F32, tag="s", name="s_ps")
                for s in range(gsz):
                    gt = o0 + ol0 + s
                    j = win_of_tile[gt]
                    jc = j - j0
                    tloc = ol0 + s
                    nc.tensor.matmul(
                        s_ps[:, s, :],
                        lhsT=vt_t[k][:, tloc * P:(tloc + 1) * P],
                        rhs=cc_t[k][:, jc * HW:(jc + 1) * HW],
                        start=True,
                        stop=True,
                    )
                sx = sxp.tile([P, GRP, H, W], BF, tag="sx", name="sx")
                sx_t[g] = sx
                nc.scalar.activation(
                    sx[:, 0:gsz].rearrange("p t h w -> p t (h w)"),
                    s_ps[:, 0:gsz, :],
                    EXP,
                )
                pt = ptp.tile([P, GRP, H, W], BF, tag="pt", name="pt")
                pt_t[g] = pt

                def mask_bcast(t0, t1):
                    mv = mk_t[k][:, (ol0 + t0) * W:(ol0 + t1) * W].rearrange(
                        "p (t w) -> p t w", w=W)
                    return AP(mv.tensor, mv.offset,
                              [mv.ap[0], mv.ap[1], [0, H], mv.ap[2]])

                # alternate mask multiplies between the otherwise-idle
                # GPSIMD (~1.15us/group) and DVE (~0.45us); strict
                # alternation keeps the slow engine from bursting past the
                # 3-group score lookahead
                eng = nc.gpsimd if g % 2 == 0 else nc.vector
                eng.tensor_mul(pt[:, 0:gsz], sx[:, 0:gsz],
                               mask_bcast(0, gsz))

            def ensure_scores(g_need):
                tgt = min(g_need + AHEAD, NG - 1)
                while state["scored"] <= tgt:
                    emit_score_group(state["scored"])
                    state["scored"] += 1

            hot = None

            def emit_quad(q):
                nonlocal hot
                wins = [4 * q + i for i in range(4)]
                last_tile = tile_off[wins[3]] + T_ws[wins[3]] - 1
                ensure_scores(grp_of_tile[last_tile][0])

                # quad accumulator: pv of 4 windows in bank0, dn in bank1
                acc = ps_acc.tile([P, 8 * HW], F32, tag="acc", name="acc")

                def pt_slice(gt):
                    g, s = grp_of_tile[gt]
                    return pt_t[g][:, s, :, :].rearrange("p h w -> p (h w)")

                # dn groups first: the reciprocal then runs on DVE while the
                # pv matmuls are still streaming on PE
                dn_last = None
                for wi, j in enumerate(wins):
                    base = (4 + wi) * HW
                    for t in range(T_ws[j]):
                        mm = nc.tensor.matmul(
                            acc[:, base:base + HW],
                            lhsT=ones_mat[:],
                            rhs=pt_slice(tile_off[j] + t),
                            start=(t == 0),
                            stop=(t == T_ws[j] - 1),
                        )
                        if t == 0 and dn_last is not None:
                            add_dep_helper(mm.ins, dn_last.ins,
                                           reason="dn group order in shared bank")
                        dn_last = mm
                rec = recp.tile([P, 4 * HW], F32, tag="rec", name="rec")
                nc.vector.reciprocal_approx_fast(rec[:], acc[:, 4 * HW:8 * HW])
                pv_last = None
                for wi, j in enumerate(wins):
                    k, ol, jc = geom[j]
                    base = wi * HW
                    for t in range(T_ws[j]):
                        mm = nc.tensor.matmul(
                            acc[:, base:base + HW],
                            lhsT=vg_t[k][:, (ol + t) * D:(ol + t + 1) * D],
                            rhs=pt_slice(tile_off[j] + t),
                            start=(t == 0),
                            stop=(t == T_ws[j] - 1),
                        )
                        if t == 0 and pv_last is not None:
                            add_dep_helper(mm.ins, pv_last.ins,
                                           reason="pv group order in shared bank")
                        pv_last = mm
                # normalize hot = pv * (1/dn) for all 4 windows at once
                qq = q % (PROJ_B // 4)
                if qq == 0:
                    hot = hotp.tile([P, PROJ_B, H, W], BF, tag="hot",
                                    name="hot")
                hslc = hot[:, qq * 4:(qq + 1) * 4].rearrange(
                    "p b h w -> p (b h w)")
                nc.vector.tensor_mul(hslc, acc[:, 0:4 * HW], rec[:])
                # output projection every PROJ_B windows
                if qq == PROJ_B // 4 - 1:
                    jbase = (q + 1) * 4 - PROJ_B
                    # ops borrows a score-PSUM slot (2 of its 4 tile slots)
                    ops = ps_s.tile([P, GRP, HW], F32, tag="s",
                                    name="ops")[:, 0:2, :].rearrange(
                                        "c b w -> c (b w)")
                    for h in range(H):
                        nc.tensor.matmul(
                            ops.rearrange("c (b w) -> c b w", b=PROJ_B),
                            lhsT=wo_sb[:, h, :],
                            rhs=hot[:, :, h, :],
                            start=(h == 0),
                            stop=(h == H - 1),
                        )
                    nc.scalar.activation(
                        outb[:, jbase * W:(jbase + PROJ_B) * W], ops, IDNT,
                        bias=bo_sb[:],
                    )

            ensure_chunk(1)
            # constants after the first chunks so they don't delay the pipe
            wo_sb = constp.tile([D, H, comb], BF, tag="wo")
            nc.sync.dma_start(wo_sb[:], wo[:].rearrange("d (h c) -> d h c", h=H))
            bo_sb = constp.tile([comb, 1], F32, tag="bo")
            nc.sync.dma_start(bo_sb[:], bo[:])
            for q in range(NW // 4):
                emit_quad(q)

            nc.sync.dma_start(out[:], outb[:])
    nc.compile()
    return nc


def _plan(ast_value, N):
    """Window/tile structure + per-edge slot assignment (core, partition, tile)."""
    E = ast_value.shape[0]
    NWg = -(-N // W)               # global window count
    order = np.argsort(ast_value, kind="stable")
    seg_s = ast_value[order].astype(np.int64)
    win_s = seg_s // W

    n_w = np.bincount(win_s, minlength=NWg)
    t_w = np.maximum(1, -(-n_w // P))
    # round-robin by descending edge count -> near-equal per-core tile budgets
    wrank = np.argsort(-n_w, kind="stable")
    core_of_w = np.empty(NWg, np.int64)
    core_of_w[wrank] = np.arange(NWg) % N_CORES
    rank_in_core = np.arange(NWg) // N_CORES    # per wrank order, desc sizes
    NW = -(-NWg // N_CORES)
    NW = -(-NW // PROJ_B) * PROJ_B
    # pair big with small: per-core rank r -> slot 2*min(r, NW-1-r) + side
    r = np.arange(NW)
    slot_of_rank = np.where(r < NW - 1 - r, 2 * r, 2 * (NW - 1 - r) + 1)
    slot_of_w = np.empty(NWg, np.int64)
    slot_of_w[wrank] = slot_of_rank[rank_in_core]
    # shared (max-over-octet) tile counts per slot; wrank desc => rank 8j max
    T_rank = np.ones(NW, np.int64)
    n_full = (NWg + N_CORES - 1) // N_CORES
    T_rank[:n_full] = t_w[wrank[0::N_CORES]]
    T_ws = np.empty(NW, np.int64)
    T_ws[slot_of_rank] = T_rank
    tile_off = np.zeros(NW + 1, np.int64)
    np.cumsum(T_ws, out=tile_off[1:])
    S_t = int(tile_off[-1])

    starts = np.zeros(NWg, np.int64)
    np.cumsum(n_w[:-1], out=starts[1:])
    rank_e = np.arange(E, dtype=np.int64) - starts[win_s]
    t_e = rank_e // P
    p_e = rank_e % P
    core_e = core_of_w[win_s]
    g_e = tile_off[slot_of_w[win_s]] + t_e
    return (order, seg_s, win_s, core_of_w, slot_of_w, NW, T_ws, tile_off,
            S_t, core_e, p_e, g_e)


def _run(ast, Wq, bq, Wk, bk, Wo, bo, ast_key, ast_value, pdg_key, pdg_value, N,
         trace=False):
    """Host orchestration: build plan from data, compile, run on 8 cores."""
    n_tbl, d = ast.shape
    assert d == D
    comb = Wo.shape[1]
    sc = 1.0 / math.sqrt(D)

    (order, seg_s, win_s, core_of_w, slot_of_w, NW, T_ws, tile_off, S_t,
     core_e, p_e, g_e) = _plan(ast_value, N)
    key_s = ast_key[order].astype(np.int64)
    NWg = -(-N // W)

    # host pre-gather: slot (core, p, tile) -> value row, in both layouts.
    # unused slots point at a zero row.
    tblz = np.vstack([ast.astype(BF16), np.zeros((1, D), BF16)])
    gidx_all = np.full((N_CORES, P, S_t), n_tbl, np.int64)
    gidx_all[core_e, p_e, g_e] = key_s
    ev_all = tblz[gidx_all.reshape(N_CORES, -1)]          # [8, P*S_t, D]
    ev_all = ev_all.reshape(N_CORES, P, S_t * D)
    evt_all = np.ascontiguousarray(
        ev_all.reshape(N_CORES, P, S_t, D).transpose(0, 3, 2, 1)
    ).reshape(N_CORES, D, S_t * P)

    # multiplicative mask
    msk_f = np.zeros((N_CORES, P, S_t * W), np.float32)
    msk_f[core_e, p_e, g_e * W + seg_s % W] = 1.0
    msk_all = msk_f.astype(BF16)

    # ---- query-side fold: C = A @ (Wq' Wk^T) + bq' @ Wk^T ----
    qsrc = np.zeros(N, np.int64)
    qsrc[pdg_key.astype(np.int64)] = pdg_value.astype(np.int64)
    A = ast[qsrc]                                        # [N, D] f32
    M = np.einsum("hij,hkj->hik", Wq * sc, Wk)           # [H, D, D]
    kap = np.einsum("hj,hkj->hk", bq * sc, Wk)           # [H, D]
    C8 = np.einsum("nd,hdk->hnk", A, M) + kap[:, None, :]  # [H, N, D]

    # per-core window lists -> cc layout [D, NW*H*W]
    wl = np.full((N_CORES, NW), -1, np.int64)
    wl[core_of_w, slot_of_w] = np.arange(NWg)
    seg_raw = wl[:, :, None] * W + np.arange(W)[None, None, :]  # [8, NW, W]
    valid = (wl[:, :, None] >= 0) & (seg_raw < N)
    seg_ids = np.clip(seg_raw, 0, N - 1)
    ccv = C8[:, seg_ids, :]                              # [H, 8, NW, W, D]
    cc_all = np.ascontiguousarray(
        ccv.transpose(1, 4, 2, 0, 3)                     # [8, D, NW, H, W]
    ).astype(BF16).reshape(N_CORES, D, NW * HW)

    # narrow-span matmuls measured SLOWER on hw (ldweights is the per-matmul
    # floor and short streams crater the DVFS duty cycle) -- full width
    wlo_t = np.zeros(S_t, np.int64)
    whi_t = np.full(S_t, W, np.int64)

    # chunks of consecutive slots with <= TC tiles, pair-aligned boundaries.
    # first chunks are small so the compute pipeline starts sooner.
    chunks = []
    j0 = 0
    tc_sched = []
    while j0 < NW:
        lim = tc_sched[len(chunks)] if len(chunks) < len(tc_sched) else TC
        j1 = j0
        while j1 < NW and tile_off[j1 + 1] - tile_off[j0] <= lim:
            j1 += 1
        if j1 < NW and (j1 - j0) % 2 == 1 and j1 - j0 >= 2:
            j1 -= 1
        chunks.append((j0, j1, int(tile_off[j0]), int(tile_off[j1])))
        j0 = j1

    wo_arr = np.ascontiguousarray(
        Wo.reshape(H, D, comb).transpose(1, 0, 2)
    ).astype(BF16).reshape(D, H * comb)
    bo_col = bo.reshape(comb, 1).astype(np.float32)

    nc = _build_nc(NW, [int(x) for x in T_ws], chunks, comb,
                   [int(x) for x in wlo_t], [int(x) for x in whi_t])
    in_maps = []
    for c in range(N_CORES):
        in_maps.append({
            "ev": ev_all[c],
            "evt": evt_all[c],
            "cc": cc_all[c],
            "msk": msk_all[c],
            "wo": wo_arr,
            "bo": bo_col,
        })
    res = bass_utils.run_bass_kernel_spmd(
        nc, in_maps, core_ids=list(range(N_CORES)), trace=trace
    )
    full = np.zeros((N, comb), np.float32)
    for c in range(N_CORES):
        outc = np.asarray(res.results[c]["out"]).astype(np.float32).T  # [S_pad, comb]
        vm = valid[c].reshape(-1)
        sel = seg_ids[c].reshape(-1)[vm]
        full[sel] = outc[: vm.shape[0]][vm]
    # empty segments: reference = bias only (p/0 is undefined there)
    seg_cnt = np.bincount(ast_value.astype(np.int64), minlength=N)[:N]
    full[seg_cnt == 0] = bo[None, :]
    return full, res


def kernel(**inputs):
    ast = np.asarray(inputs["ast_nodes_encodings"], np.float32)
    Wq = np.asarray(inputs["Wq"], np.float32)
    bq = np.asarray(inputs["bq"], np.float32)
    Wk = np.asarray(inputs["Wk"], np.float32)
    bk = np.asarray(inputs["bk"], np.float32)  # cancels inside segment softmax
    Wo = np.asarray(inputs["Wo"], np.float32)
    bo = np.asarray(inputs["bo"], np.float32)
    ast_key = np.asarray(inputs["ast_key"]).astype(np.int64)
    ast_value = np.asarray(inputs["ast_value"]).astype(np.int64)
    pdg_key = np.asarray(inputs["pdg_key"]).astype(np.int64)
    pdg_value = np.asarray(inputs["pdg_value"]).astype(np.int64)
    N = int(np.asarray(inputs["nr_cfg_nodes"]))
    out, _ = _run(ast, Wq, bq, Wk, bk, Wo, bo,
                  ast_key, ast_value, pdg_key, pdg_value, N)
    return out
